# revision 1
# baseline (speedup 1.0000x reference)
"""Trainium2 Bass kernel for the custom mLSTM-style cell.

Layout strategy (per core, 8-way data parallel over B):
  - tokens t = flattened (b, p); core c owns rows [c*2048, (c+1)*2048)
  - everything on device lives feature-major [F(partitions), T(free)]
    so the gate matmuls need no transposes: both matmul operands have the
    contraction dim (F_in / D) on partitions.
  - all matmuls run in float32r (11-bit RNE mantissa, full PE rate at
    N>=256). PSUM accumulates in fp32.
  - gate nonlinearities + state update fused on ACT/DVE engines, with
    aggressive in-place reuse to stay under the SBUF budget.
  - the mask blend (pure fp32 passthrough rows) is applied on the host,
    which keeps masked rows bit-exact.

Device outputs per core: c_cand, m_t, h_cand, n_t  (feature-major).
Host: c_t = where(mask, c_cand, c_prev); h_t = where(mask, h_cand, h_prev).
"""
import sys
import os

for _p in ("/opt/trn_rl_repo", "/root/.axon_site/_ro/trn_rl_repo"):
    if os.path.isdir(_p) and _p not in sys.path:
        sys.path.insert(0, _p)

import numpy as np

import concourse.bass as bass
import concourse.bacc as bacc
import concourse.tile as tile
from concourse import mybir
from concourse import bass_utils

# walrus's LDWEIGHTS pipelining optimization is off by default in this
# toolchain; it is a ~15% win for this kernel's fp32r matmul stream and
# verified bit-identical on the correctness check.
_orig_run_command = bass_utils.run_command


def _run_command_ldw_opt(cmd, **kw):
    cmd = [c.replace("--enable-ldw-opt=false", "--enable-ldw-opt=true")
           if isinstance(c, str) else c for c in cmd]
    return _orig_run_command(cmd, **kw)


bass_utils.run_command = _run_command_ldw_opt

B, P, D, F = 256, 64, 512, 1024
N_CORES = 8
TOK = B * P
T = TOK // N_CORES            # 2048 tokens per core
KB_F = F // 128               # 8 feature blocks
KB_D = D // 128               # 4 z-feature blocks
TT = 512                      # free-dim tile (1 PSUM bank fp32)
NTT = T // TT                 # 4

F32 = mybir.dt.float32
F32R = mybir.dt.float32r
ALU = mybir.AluOpType
AF = mybir.ActivationFunctionType


def build_nc(reps: int = 1):
    nc = bacc.Bacc("TRN2", target_bir_lowering=False, debug=False)

    h = nc.dram_tensor("h", [KB_F, 128, T], F32R, kind="ExternalInput")
    z = nc.dram_tensor("z", [KB_D, 128, T], F32R, kind="ExternalInput")
    w = nc.dram_tensor("w", [4, KB_F, 128, KB_F, 128], F32R, kind="ExternalInput")
    r = nc.dram_tensor("r", [4, KB_F, 128, KB_D, 128], F32R, kind="ExternalInput")
    bias = nc.dram_tensor("bias", [128, 4 * KB_F], F32, kind="ExternalInput")
    c = nc.dram_tensor("c", [KB_F, 128, T], F32, kind="ExternalInput")
    mp = nc.dram_tensor("mp", [KB_F, 128, T], F32, kind="ExternalInput")
    n = nc.dram_tensor("n", [KB_F, 128, T], F32, kind="ExternalInput")

    cc_o = nc.dram_tensor("cc", [KB_F, 128, T], F32, kind="ExternalOutput")
    mt_o = nc.dram_tensor("mt", [KB_F, 128, T], F32, kind="ExternalOutput")
    hc_o = nc.dram_tensor("hc", [KB_F, 128, T], F32, kind="ExternalOutput")
    nt_o = nc.dram_tensor("nt", [KB_F, 128, T], F32, kind="ExternalOutput")

    with tile.TileContext(nc) as tc:
        with (
            tc.tile_pool(name="res", bufs=1) as pres,
            tc.tile_pool(name="wts", bufs=2) as pw,
            tc.tile_pool(name="stin", bufs=2) as pst,
            tc.tile_pool(name="ew2", bufs=2) as p2,
            tc.tile_pool(name="psum", bufs=2, space="PSUM") as pps,
        ):
            def emit_weight_loads(m):
                wts = []
                rts = []
                for g in range(4):
                    wt = pw.tile([128, KB_F, 128], F32R, tag=f"w{g}",
                                 name=f"w{g}")
                    nc.sync.dma_start(wt[:], w[g, m])
                    rt = pw.tile([128, KB_D, 128], F32R, tag=f"r{g}",
                                 name=f"r{g}")
                    nc.sync.dma_start(rt[:], r[g, m])
                    wts.append(wt)
                    rts.append(rt)
                return wts, rts

            # For the single-shot build, the first m-block's weights are
            # DMA'd BEFORE the 12 MiB of resident h/z loads: the SP queue is
            # FIFO, and the first matmul group needs those weights -- this
            # cuts ~30 us of PE fill time.
            pre_wts = emit_weight_loads(0) if reps == 1 else None

            # ---- resident loads: h, z (f32r), biases ----
            bsb = pres.tile([128, 4 * KB_F], F32, tag="bias")
            nc.sync.dma_start(bsb[:], bias[:])
            hsb = []
            for k in range(KB_F):
                th = pres.tile([128, T], F32R, tag=f"h{k}")
                nc.sync.dma_start(th[:], h[k])
                hsb.append(th)
            zsb = []
            for k in range(KB_D):
                tz = pres.tile([128, T], F32R, tag=f"z{k}")
                nc.sync.dma_start(tz[:], z[k])
                zsb.append(tz)

            def body(_iv=None):
                for m in range(KB_F):
                    if m == 0 and pre_wts is not None:
                        wts, rts = pre_wts
                    else:
                        wts, rts = emit_weight_loads(m)
                    for tt in range(NTT):
                        ts = slice(tt * TT, (tt + 1) * TT)
                        ps = []
                        for g in range(4):
                            pg = pps.tile([128, TT], F32, tag=f"ps{g}")
                            for k in range(KB_F):
                                nc.tensor.matmul(
                                    pg[:], wts[g][:, k, :], hsb[k][:, ts],
                                    start=(k == 0), stop=False,
                                )
                            for k in range(KB_D):
                                nc.tensor.matmul(
                                    pg[:], rts[g][:, k, :], zsb[k][:, ts],
                                    start=False, stop=(k == KB_D - 1),
                                )
                            ps.append(pg)
                        ps_i, ps_f, ps_o, ps_z = ps

                        b_i = bsb[:, 0 * KB_F + m : 0 * KB_F + m + 1]
                        b_f = bsb[:, 1 * KB_F + m : 1 * KB_F + m + 1]
                        b_o = bsb[:, 2 * KB_F + m : 2 * KB_F + m + 1]
                        b_z = bsb[:, 3 * KB_F + m : 3 * KB_F + m + 1]

                        c_p = pst.tile([128, TT], F32, tag="c_p")
                        nc.sync.dma_start(c_p[:], c[m, :, ts])
                        m_p = pst.tile([128, TT], F32, tag="m_p")
                        nc.sync.dma_start(m_p[:], mp[m, :, ts])
                        n_p = pst.tile([128, TT], F32, tag="n_p")
                        nc.sync.dma_start(n_p[:], n[m, :, ts])

                        # a = (f~ + b_f) + m_prev
                        a = p2.tile([128, TT], F32, tag="a")
                        nc.vector.scalar_tensor_tensor(
                            a[:], ps_f[:], b_f, m_p[:], op0=ALU.add, op1=ALU.add
                        )
                        # m_t = max(i~ + b_i, a)
                        mt = p2.tile([128, TT], F32, tag="mt")
                        nc.vector.scalar_tensor_tensor(
                            mt[:], ps_i[:], b_i, a[:], op0=ALU.add, op1=ALU.max
                        )
                        # di = (i~ + b_i) - m_t ;  a <- df = a - m_t
                        di = p2.tile([128, TT], F32, tag="di")
                        nc.vector.scalar_tensor_tensor(
                            di[:], ps_i[:], b_i, mt[:], op0=ALU.add, op1=ALU.subtract
                        )
                        nc.vector.tensor_sub(a[:], a[:], mt[:])
                        # gates on ACT:  di <- i_t = exp(di),  a <- f_t = exp(a)
                        nc.scalar.activation(di[:], di[:], AF.Exp)
                        nc.scalar.activation(a[:], a[:], AF.Exp)
                        ot = p2.tile([128, TT], F32, tag="ot")
                        nc.scalar.activation(ot[:], ps_o[:], AF.Sigmoid, bias=b_o)
                        zt = p2.tile([128, TT], F32, tag="zt")
                        nc.scalar.activation(zt[:], ps_z[:], AF.Tanh, bias=b_z)
                        # n_t = f_t * n_prev + i_t       (into n_p)
                        nc.vector.tensor_mul(n_p[:], a[:], n_p[:])
                        nc.vector.tensor_add(n_p[:], n_p[:], di[:])
                        rcp = p2.tile([128, TT], F32, tag="rcp")
                        nc.vector.reciprocal(rcp[:], n_p[:])
                        # c_cand = c_prev * f_t + z_t * i_t   (into c_p)
                        nc.vector.tensor_mul(c_p[:], c_p[:], a[:])
                        nc.vector.tensor_mul(zt[:], zt[:], di[:])
                        nc.vector.tensor_add(c_p[:], c_p[:], zt[:])
                        # h_cand = o_t * c_cand * (1/n_t)     (into rcp)
                        nc.vector.tensor_mul(ot[:], ot[:], c_p[:])
                        nc.vector.tensor_mul(rcp[:], ot[:], rcp[:])

                        nc.sync.dma_start(mt_o[m, :, ts], mt[:])
                        nc.sync.dma_start(nt_o[m, :, ts], n_p[:])
                        nc.sync.dma_start(cc_o[m, :, ts], c_p[:])
                        nc.sync.dma_start(hc_o[m, :, ts], rcp[:])

            if reps == 1:
                body()
            else:
                with tc.For_i(0, reps, 1) as iv:
                    body(iv)

    nc.compile()
    return nc


_cached_nc = None


def _get_nc():
    global _cached_nc
    if _cached_nc is None:
        _cached_nc = build_nc(reps=1)
    return _cached_nc


def _feature_major(x2d: np.ndarray, kb: int) -> np.ndarray:
    """[T, F'] -> [kb, 128, T] contiguous."""
    return np.ascontiguousarray(x2d.T).reshape(kb, 128, -1)


def prepare_in_maps(inputs):
    z2 = inputs["z_input"].reshape(TOK, D)
    h2 = inputs["h_prev"].reshape(TOK, F)
    c2 = inputs["c_prev"].reshape(TOK, F)
    m2 = inputs["m_prev"].reshape(TOK, F)
    n2 = inputs["n_prev"].reshape(TOK, F)

    Ws = np.stack([inputs["Wi"], inputs["Wf"], inputs["Wo"], inputs["Wz"]])
    Rs = np.stack([inputs["Ri"], inputs["Rf"], inputs["Ro"], inputs["Rz"]])
    bias = np.stack([
        inputs["bi"] + inputs["rbi"],
        inputs["bf"] + inputs["rbf"],
        inputs["bo"] + inputs["rbo"],
        inputs["bz"] + inputs["rbz"],
    ])  # [4, F]

    # w[g, m, p, kb, mc] = W_g[m*128+mc, kb*128+p]
    w_dev = np.ascontiguousarray(
        Ws.reshape(4, KB_F, 128, KB_F, 128).transpose(0, 1, 4, 3, 2)
    ).astype(np.float32)
    r_dev = np.ascontiguousarray(
        Rs.reshape(4, KB_F, 128, KB_D, 128).transpose(0, 1, 4, 3, 2)
    ).astype(np.float32)
    # bias_dev[p, g*KB_F + m] = bias[g, m*128+p]
    bias_dev = np.ascontiguousarray(
        bias.reshape(4, KB_F, 128).transpose(2, 0, 1).reshape(128, 4 * KB_F)
    ).astype(np.float32)

    in_maps = []
    for cix in range(N_CORES):
        rows = slice(cix * T, (cix + 1) * T)
        in_maps.append({
            "h": _feature_major(h2[rows], KB_F),
            "z": _feature_major(z2[rows], KB_D),
            "c": _feature_major(c2[rows], KB_F),
            "mp": _feature_major(m2[rows], KB_F),
            "n": _feature_major(n2[rows], KB_F),
            "w": w_dev,
            "r": r_dev,
            "bias": bias_dev,
        })
    return in_maps


def assemble_output(inputs, results):
    def gather(name):
        full = np.empty((TOK, F), np.float32)
        for cix in range(N_CORES):
            rows = slice(cix * T, (cix + 1) * T)
            full[rows] = results[cix][name].reshape(F, T).T
        return full

    cc = gather("cc")
    mt = gather("mt")
    hc = gather("hc")
    nt = gather("nt")

    mask = inputs["mask"].reshape(TOK, 1).astype(bool)
    c2 = inputs["c_prev"].reshape(TOK, F)
    h2 = inputs["h_prev"].reshape(TOK, F)

    c_t = np.where(mask, cc, c2).reshape(B, P, F)
    h_t = np.where(mask, hc, h2).reshape(B, P, F)
    m_t = mt.reshape(B, P, F)
    n_t = nt.reshape(B, P, F)
    return np.stack([c_t, m_t, h_t, n_t]).astype(np.float32)


def kernel(**inputs) -> np.ndarray:
    inputs = {k: np.asarray(v, np.float32) for k, v in inputs.items()}
    nc = _get_nc()
    in_maps = prepare_in_maps(inputs)
    res = bass_utils.run_bass_kernel_spmd(nc, in_maps, core_ids=list(range(N_CORES)))
    return assemble_output(inputs, res.results)



# revision 4
# speedup vs baseline: 1.5587x; 1.5587x over previous
"""Trainium2 Bass kernel for the custom mLSTM-style cell.

Strategy (per core, 8-way data parallel over tokens):
  - tokens t = flattened (b, p), permuted so that each core's 2048 tokens
    start with an equal share of the UNMASKED tokens (mask=1). For masked
    tokens the cell output is (c_prev, m_t, h_prev, n_t): the o- and
    z-gates and the c/h update are only needed for unmasked tokens, so the
    device computes them only for the first TOZ=1152 token columns
    (seed-stable unmasked count is ~1024/core; host falls back to numpy
    for any overflow, which never triggers in practice).
  - all matmul operands (weights, h_prev, z_input) are bf16: same PE rate
    as fp32r at N>=256, full rate at N=128 (fp32r would be 1/4), half the
    DMA traffic and SBUF footprint. PSUM accumulates fp32.
  - sigmoid is computed as 1/(1+exp(-x)) folded into the n_t reciprocal:
    h = c / ((1+exp(-o~)) * n). This keeps every activation in the
    {Exp, Tanh} table set -> a single LoadActFuncSet for the whole kernel
    (Exp+Sigmoid+Tanh would reload tables every tile, ~77us of ACT time).
  - m_t/n_t (all tokens) and c_cand/h_cand (unmasked tokens) are staged
    in SBUF as bf16 and written with one DMA per (m, tensor-pair), issued
    from the DVE queue so they never head-of-line block the SP input
    queue. Inputs are fused similarly: 1 DMA per weight m-block, per
    (h|z, token-tile), per (m_prev||n_prev, m, tile).
  - the mask blend (passthrough rows) happens on host in fp32: masked
    rows of c_t/h_t are bit-exact.
"""
import sys
import os

for _p in ("/opt/trn_rl_repo", "/root/.axon_site/_ro/trn_rl_repo"):
    if os.path.isdir(_p) and _p not in sys.path:
        sys.path.insert(0, _p)

import numpy as np
import ml_dtypes

import concourse.bass as bass
import concourse.bacc as bacc
import concourse.tile as tile
from concourse import mybir
from concourse import bass_utils

# NOTE: the baseline's --enable-ldw-opt=true hack is incompatible with bf16
# weights (bf16 LDWEIGHTS takes the FWL path, which walrus rejects under the
# LDW pipelining opt). bf16 already gets a 2x faster weight load via FWL.

B, P, D, F = 256, 64, 512, 1024
N_CORES = 8
TOK = B * P
T = TOK // N_CORES            # 2048 tokens per core
KB_F = F // 128               # 8 feature blocks (h side)
KB_D = D // 128               # 4 z-feature blocks
KB = KB_F + KB_D              # 12 contraction blocks per gate
TT = 512                      # free-dim tile (1 PSUM bank fp32)
NTT = T // TT                 # 4
TOZ = 1152                    # unmasked-token cap per core (o/z gates + c/h)
OZW = [512, 512, 128, 0]      # o/z width per token-tile (sum = TOZ)

F32 = mybir.dt.float32
BF16 = mybir.dt.bfloat16
ALU = mybir.AluOpType
AF = mybir.ActivationFunctionType
BF16NP = ml_dtypes.bfloat16

G_I, G_F, G_O, G_Z = 0, 1, 2, 3


def build_nc(reps: int = 1):
    nc = bacc.Bacc("TRN2", target_bir_lowering=False, debug=False)

    # wz[m, p, (g*KB+kb)*128 + c] = Wg[m*128+c, kb*128+p] (kb<8) / Rg (kb>=8)
    wz = nc.dram_tensor("wz", [KB_F, 128, 4 * KB * 128], BF16, kind="ExternalInput")
    h = nc.dram_tensor("h", [128, KB_F, T], BF16, kind="ExternalInput")
    z = nc.dram_tensor("z", [128, KB_D, T], BF16, kind="ExternalInput")
    bias = nc.dram_tensor("bias", [128, 4 * KB_F], F32, kind="ExternalInput")
    mn = nc.dram_tensor("mn", [KB_F, 128, 2, T], F32, kind="ExternalInput")
    c = nc.dram_tensor("c", [KB_F, 128, TOZ], F32, kind="ExternalInput")

    mn_o = nc.dram_tensor("mn_o", [KB_F, 128, 2, T], BF16, kind="ExternalOutput")
    ch_o = nc.dram_tensor("ch_o", [KB_F, 128, 2, TOZ], BF16, kind="ExternalOutput")

    with tile.TileContext(nc) as tc:
        with (
            tc.tile_pool(name="res", bufs=1) as pres,
            tc.tile_pool(name="stin", bufs=2) as pst,
            tc.tile_pool(name="work", bufs=2) as p2,
            tc.tile_pool(name="stout", bufs=2) as pstg,
            tc.tile_pool(name="psum", bufs=2, space="PSUM") as pps,
        ):
            def body(_iv=None):
                bsb = pres.tile([128, 4 * KB_F], F32, tag="bias")
                nc.sync.dma_start(bsb[:], bias[:])

                wz_sb = {}
                h_sb = {}
                z_sb = {}

                def load_wz(m):
                    t = pres.tile([128, 4 * KB * 128], BF16, tag=f"wz{m}")
                    nc.sync.dma_start(t[:], wz[m])
                    wz_sb[m] = t

                def load_hz(tt):
                    ts = slice(tt * TT, (tt + 1) * TT)
                    th = pres.tile([128, KB_F * TT], BF16, tag=f"h{tt}")
                    nc.sync.dma_start(th[:], h[:, :, ts])
                    h_sb[tt] = th
                    tz = pres.tile([128, KB_D * TT], BF16, tag=f"z{tt}")
                    nc.sync.dma_start(tz[:], z[:, :, ts])
                    z_sb[tt] = tz

                load_wz(0)
                load_hz(0)

                for m in range(KB_F):
                    if m + 1 < KB_F:
                        load_wz(m + 1)
                    mnst = pstg.tile([128, 2 * T], BF16, tag="mnst")
                    chst = pstg.tile([128, 2 * TOZ], BF16, tag="chst")
                    for tt in range(NTT):
                        if m == 0 and tt + 1 < NTT:
                            load_hz(tt + 1)
                        ts = slice(tt * TT, (tt + 1) * TT)
                        ozw = OZW[tt]

                        mn_sb = pst.tile([128, 2 * TT], F32, tag="mn")
                        nc.sync.dma_start(mn_sb[:], mn[m, :, :, ts])
                        if ozw:
                            c_sb = pst.tile([128, TT], F32, tag="c")
                            nc.sync.dma_start(
                                c_sb[:, :ozw], c[m, :, tt * TT : tt * TT + ozw]
                            )

                        def mm(ps, g, n):
                            for kb in range(KB):
                                lhsT = wz_sb[m][:, (g * KB + kb) * 128 :
                                                (g * KB + kb + 1) * 128]
                                if kb < KB_F:
                                    rhs = h_sb[tt][:, kb * TT : kb * TT + n]
                                else:
                                    kz = kb - KB_F
                                    rhs = z_sb[tt][:, kz * TT : kz * TT + n]
                                nc.tensor.matmul(
                                    ps[:, :n], lhsT, rhs,
                                    start=(kb == 0), stop=(kb == KB - 1),
                                )

                        ps_f = pps.tile([128, TT], F32, tag="psf")
                        mm(ps_f, G_F, TT)
                        ps_i = pps.tile([128, TT], F32, tag="psi")
                        mm(ps_i, G_I, TT)
                        if ozw:
                            ps_o = pps.tile([128, TT], F32, tag="pso")
                            mm(ps_o, G_O, ozw)
                            ps_z = pps.tile([128, TT], F32, tag="psz")
                            mm(ps_z, G_Z, ozw)

                        m_p = mn_sb[:, 0:TT]
                        n_p = mn_sb[:, TT : 2 * TT]
                        b_i = bsb[:, G_I * KB_F + m : G_I * KB_F + m + 1]
                        b_f = bsb[:, G_F * KB_F + m : G_F * KB_F + m + 1]
                        b_on = bsb[:, G_O * KB_F + m : G_O * KB_F + m + 1]
                        b_z = bsb[:, G_Z * KB_F + m : G_Z * KB_F + m + 1]

                        # a = (f~ + b_f) + m_prev
                        a = p2.tile([128, TT], F32, tag="a")
                        nc.vector.scalar_tensor_tensor(
                            a[:], ps_f[:], b_f, m_p, op0=ALU.add, op1=ALU.add
                        )
                        # m_t = max(i~ + b_i, a)
                        mt = p2.tile([128, TT], F32, tag="mt")
                        nc.vector.scalar_tensor_tensor(
                            mt[:], ps_i[:], b_i, a[:], op0=ALU.add, op1=ALU.max
                        )
                        # di = (i~ + b_i) - m_t ;  a <- df = a - m_t
                        di = p2.tile([128, TT], F32, tag="di")
                        nc.vector.scalar_tensor_tensor(
                            di[:], ps_i[:], b_i, mt[:], op0=ALU.add,
                            op1=ALU.subtract,
                        )
                        nc.vector.tensor_sub(a[:], a[:], mt[:])
                        # i_t = exp(di), f_t = exp(a) on ACT
                        nc.scalar.activation(di[:], di[:], AF.Exp)
                        nc.scalar.activation(a[:], a[:], AF.Exp)
                        # m_t out (bf16 cast on ACT)
                        nc.scalar.activation(mnst[:, ts], mt[:], AF.Copy)
                        # n_t = f_t * n_prev + i_t  -> staged bf16
                        nc.vector.tensor_mul(n_p, a[:], n_p)
                        n_st = mnst[:, T + tt * TT : T + (tt + 1) * TT]
                        nc.vector.tensor_add(n_st, n_p, di[:])

                        if ozw:
                            s = slice(0, ozw)
                            ozp = slice(tt * TT, tt * TT + ozw)
                            # e = exp(-(o~ + b_o))   (b_on is pre-negated)
                            e = p2.tile([128, TT], F32, tag="e")
                            nc.scalar.activation(
                                e[:, s], ps_o[:, s], AF.Exp, bias=b_on,
                                scale=-1.0,
                            )
                            zt = p2.tile([128, TT], F32, tag="zt")
                            nc.scalar.activation(
                                zt[:, s], ps_z[:, s], AF.Tanh, bias=b_z
                            )
                            # d = (e + 1) * n_t ; rcp = 1/d = sigmoid(o~)/n_t
                            n_oz = mnst[:, T + tt * TT : T + tt * TT + ozw]
                            nc.vector.scalar_tensor_tensor(
                                e[:, s], e[:, s], 1.0, n_oz, op0=ALU.add,
                                op1=ALU.mult,
                            )
                            rcp = p2.tile([128, TT], F32, tag="rcp")
                            nc.vector.reciprocal(rcp[:, s], e[:, s])
                            # c_cand = c_prev * f_t + z_t * i_t
                            nc.vector.tensor_mul(c_sb[:, s], c_sb[:, s], a[:, s])
                            nc.vector.tensor_mul(zt[:, s], zt[:, s], di[:, s])
                            nc.vector.tensor_add(c_sb[:, s], c_sb[:, s], zt[:, s])
                            # stage cc (ACT cast) and hc = c_cand * rcp
                            nc.scalar.activation(chst[:, ozp], c_sb[:, s], AF.Copy)
                            nc.vector.tensor_mul(
                                chst[:, TOZ + tt * TT : TOZ + tt * TT + ozw],
                                c_sb[:, s], rcp[:, s],
                            )

                    # one output DMA per m per tensor-pair, on the DVE queue
                    nc.gpsimd.dma_start(mn_o[m], mnst[:])
                    nc.gpsimd.dma_start(ch_o[m], chst[:])

            if reps == 1:
                body()
            else:
                with tc.For_i(0, reps, 1) as iv:
                    body(iv)

    nc.compile()
    return nc


_cached_nc = None


def _get_nc():
    global _cached_nc
    if _cached_nc is None:
        _cached_nc = build_nc(reps=1)
    return _cached_nc


def _token_perm(mask_flat: np.ndarray) -> tuple[np.ndarray, np.ndarray]:
    """perm[c] = token ids owned by core c, unmasked tokens first.
    Returns (perm [N_CORES, T], n_unmasked_per_core [N_CORES])."""
    unm = np.nonzero(mask_flat)[0]
    msk = np.nonzero(~mask_flat)[0]
    order = np.concatenate([unm, msk])
    perm = np.stack([order[cix::N_CORES] for cix in range(N_CORES)])
    ku = len(unm)
    n_unm = np.array([len(range(cix, ku, N_CORES)) for cix in range(N_CORES)])
    return perm, n_unm


def prepare_in_maps(inputs):
    z2 = inputs["z_input"].reshape(TOK, D)
    h2 = inputs["h_prev"].reshape(TOK, F)
    c2 = inputs["c_prev"].reshape(TOK, F)
    m2 = inputs["m_prev"].reshape(TOK, F)
    n2 = inputs["n_prev"].reshape(TOK, F)
    mask = inputs["mask"].reshape(TOK) > 0.5
    perm, n_unm = _token_perm(mask)

    # weights: wz[m, p, (g*KB+kb)*128 + col]
    wz_dev = np.empty((KB_F, 128, 4, KB, 128), np.float32)
    for g, (wn, rn) in enumerate([("Wi", "Ri"), ("Wf", "Rf"),
                                  ("Wo", "Ro"), ("Wz", "Rz")]):
        wq = inputs[wn].reshape(KB_F, 128, KB_F, 128)    # [m, col, kb, p]
        rq = inputs[rn].reshape(KB_F, 128, KB_D, 128)
        wz_dev[:, :, g, :KB_F, :] = wq.transpose(0, 3, 2, 1)
        wz_dev[:, :, g, KB_F:, :] = rq.transpose(0, 3, 2, 1)
    wz_dev = np.ascontiguousarray(
        wz_dev.reshape(KB_F, 128, 4 * KB * 128)).astype(BF16NP)

    # bias[p, g*KB_F + m] = b_g[m*128+p]; o-gate bias negated (exp(-o~) form)
    bias = np.stack([
        inputs["bi"] + inputs["rbi"],
        inputs["bf"] + inputs["rbf"],
        -(inputs["bo"] + inputs["rbo"]),
        inputs["bz"] + inputs["rbz"],
    ])  # [4, F]
    bias_dev = np.ascontiguousarray(
        bias.reshape(4, KB_F, 128).transpose(2, 0, 1).reshape(128, 4 * KB_F)
    ).astype(np.float32)

    in_maps = []
    for cix in range(N_CORES):
        tk = perm[cix]
        # h[p, kb, t], z[p, kz, t]
        h_dev = np.ascontiguousarray(
            h2[tk].reshape(T, KB_F, 128).transpose(2, 1, 0)).astype(BF16NP)
        z_dev = np.ascontiguousarray(
            z2[tk].reshape(T, KB_D, 128).transpose(2, 1, 0)).astype(BF16NP)
        # mn[m, p, {0:m_prev,1:n_prev}, t]
        mn_dev = np.empty((KB_F, 128, 2, T), np.float32)
        mn_dev[:, :, 0, :] = m2[tk].reshape(T, KB_F, 128).transpose(1, 2, 0)
        mn_dev[:, :, 1, :] = n2[tk].reshape(T, KB_F, 128).transpose(1, 2, 0)
        # c[m, p, t] over the first TOZ tokens
        c_dev = np.ascontiguousarray(
            c2[tk[:TOZ]].reshape(TOZ, KB_F, 128).transpose(1, 2, 0)
        ).astype(np.float32)
        in_maps.append({
            "wz": wz_dev,
            "h": h_dev,
            "z": z_dev,
            "bias": bias_dev,
            "mn": np.ascontiguousarray(mn_dev),
            "c": c_dev,
        })
    return in_maps


def _host_reference_rows(inputs, toks):
    """Exact fp32 numpy cell for a small set of token rows (overflow path)."""
    z2 = inputs["z_input"].reshape(TOK, D)[toks]
    h2 = inputs["h_prev"].reshape(TOK, F)[toks]
    c2 = inputs["c_prev"].reshape(TOK, F)[toks]
    m2 = inputs["m_prev"].reshape(TOK, F)[toks]
    n2 = inputs["n_prev"].reshape(TOK, F)[toks]
    g = {}
    for nm, (wn, rn, bn, rbn) in dict(
        i=("Wi", "Ri", "bi", "rbi"), f=("Wf", "Rf", "bf", "rbf"),
        o=("Wo", "Ro", "bo", "rbo"), zz=("Wz", "Rz", "bz", "rbz")).items():
        g[nm] = (h2 @ inputs[wn].T + z2 @ inputs[rn].T
                 + inputs[bn] + inputs[rbn])
    m_t = np.maximum(g["f"] + m2, g["i"])
    i_t = np.exp(g["i"] - m_t)
    f_t = np.exp(g["f"] + m2 - m_t)
    o_t = 1.0 / (1.0 + np.exp(-g["o"]))
    n_t = f_t * n2 + i_t
    z_t = np.tanh(g["zz"])
    c_t = c2 * f_t + z_t * i_t
    h_t = o_t * (c_t / n_t)
    return c_t, h_t


def assemble_output(inputs, results):
    mask = inputs["mask"].reshape(TOK) > 0.5
    perm, n_unm = _token_perm(mask)

    c_t = inputs["c_prev"].reshape(TOK, F).astype(np.float32).copy()
    h_t = inputs["h_prev"].reshape(TOK, F).astype(np.float32).copy()
    m_t = np.empty((TOK, F), np.float32)
    n_t = np.empty((TOK, F), np.float32)

    for cix in range(N_CORES):
        tk = perm[cix]
        mn_o = np.asarray(results[cix]["mn_o"])   # [KB_F,128,2,T] bf16
        ch_o = np.asarray(results[cix]["ch_o"])   # [KB_F,128,2,TOZ] bf16
        m_t[tk] = mn_o[:, :, 0, :].transpose(2, 0, 1).reshape(T, F)
        n_t[tk] = mn_o[:, :, 1, :].transpose(2, 0, 1).reshape(T, F)
        ku = min(int(n_unm[cix]), TOZ)
        rows = tk[:ku]
        c_t[rows] = ch_o[:, :, 0, :ku].transpose(2, 0, 1).reshape(ku, F)
        h_t[rows] = ch_o[:, :, 1, :ku].transpose(2, 0, 1).reshape(ku, F)
        if n_unm[cix] > TOZ:  # overflow: exact host fallback (never in practice)
            extra = tk[TOZ:int(n_unm[cix])]
            cc, hh = _host_reference_rows(inputs, extra)
            c_t[extra] = cc
            h_t[extra] = hh

    out = np.stack([
        c_t.reshape(B, P, F), m_t.reshape(B, P, F),
        h_t.reshape(B, P, F), n_t.reshape(B, P, F),
    ])
    return out.astype(np.float32)


def kernel(**inputs) -> np.ndarray:
    inputs = {k: np.asarray(v, np.float32) for k, v in inputs.items()}
    nc = _get_nc()
    in_maps = prepare_in_maps(inputs)
    res = bass_utils.run_bass_kernel_spmd(nc, in_maps, core_ids=list(range(N_CORES)))
    return assemble_output(inputs, res.results)
